# revision 1
# baseline (speedup 1.0000x reference)
"""Trainium2 Bass kernel for nn_AttLayer_9972914061697 (sparse_attention).

Reference computation (jax):
    q, k, v = split(x, 3, axis=-1)              # x: [B=4, T=4096, 3C=384]
    score   = einsum('btc,bsc->bts', k, q) / sqrt(C)
    out     = softmax(score, -1) @ v            # [B, T, C=128]

Sharding: 8 cores = 4 batches x 2 T-halves (data parallel, zero comm).
Each core holds the full q, v of its batch plus its 2048-row k chunk and
produces its 2048-row output chunk.

Per-core algorithm (all matmuls bf16, accumulation f32):
  - transpose q, k on-chip (PE transpose) into [C, T] layout
  - S_T[s, t] = sum_c q[s,c] k[t,c] computed as qT_chunk.T @ kT (PSUM f32)
  - P_T = exp(S_T / sqrt(C)) via ScalarE, written bf16 to SBUF
  - out[t, 0:128] & rowsum[t] in one PSUM accumulation: P_T_chunk.T @ [v | 1]
    (ones column appended to v makes the softmax denominator an extra column)
  - out = out * 1/rowsum (VectorE reciprocal + per-partition scalar mul)
"""

import numpy as np

import concourse.bass as bass
import concourse.tile as tile
from concourse import bacc, mybir
from concourse.bass_utils import run_bass_kernel_spmd
from concourse.masks import make_identity

F32 = mybir.dt.float32
BF16 = mybir.dt.bfloat16

B = 4
T = 4096
C = 128
N_CORES = 8
TL = T // 2          # 2048 t-rows per core
NSC = T // 128       # 32 s-chunks
NTT = TL // 128      # 16 t-tiles
GSC = 8              # s-chunks per group (PV flush granularity)
NG = NSC // GSC      # 4 groups
SCALE = 1.0 / float(np.sqrt(C))
LB = 4               # 128-row tiles per input DMA


def build_nc():
    nc = bacc.Bacc()
    q = nc.declare_dram_parameter("q", [T, C], F32, isOutput=False)
    k = nc.declare_dram_parameter("k", [TL, C], F32, isOutput=False)
    v = nc.declare_dram_parameter("v", [T, C], F32, isOutput=False)
    out = nc.declare_dram_parameter("out", [TL, C], F32, isOutput=True)

    qv = q[:].rearrange("(n p) c -> p n c", p=128)    # [128, 32, 128]
    kv = k[:].rearrange("(n p) c -> p n c", p=128)    # [128, 16, 128]
    vw = v[:].rearrange("(n p) c -> p n c", p=128)    # [128, 32, 128]
    ov = out[:].rearrange("(n p) c -> p n c", p=128)  # [128, 16, 128]

    with tile.TileContext(nc) as tc:
        with (
            tc.tile_pool(name="const", bufs=1) as const_pool,
            tc.tile_pool(name="qkt", bufs=1) as qkt_pool,
            tc.tile_pool(name="vbuf", bufs=1) as v_pool,
            tc.tile_pool(name="stage", bufs=4) as stage_pool,
            tc.tile_pool(name="pT", bufs=2) as pT_pool,
            tc.tile_pool(name="oacc", bufs=1) as oacc_pool,
            tc.tile_pool(name="ost", bufs=2) as ost_pool,
            tc.tile_pool(name="spsum", bufs=3, space="PSUM") as spsum,
            tc.tile_pool(name="opsum", bufs=2, space="PSUM") as opsum,
        ):
            identity = const_pool.tile([128, 128], F32)
            make_identity(nc, identity[:])

            qT = qkt_pool.tile([128, T], BF16, tag="qT")     # q transposed [c, s]
            kT = qkt_pool.tile([128, TL], BF16, tag="kT")    # k transposed [c, t]
            vv = v_pool.tile([128, NSC * (C + 1)], BF16)     # 32 x [128, 129] chunks
            vv3 = vv[:].rearrange("p (n c) -> p n c", c=C + 1)
            oacc = oacc_pool.tile([128, NTT * (C + 1)], F32)
            oacc3 = oacc[:].rearrange("p (n c) -> p n c", c=C + 1)
            rtile = const_pool.tile([128, NTT], F32, tag="recip")

            # ones column of every v chunk
            nc.vector.memset(vv3[:, :, C : C + 1], 1.0)

            # ---- load + transpose q into qT, k into kT (groups of 8 tiles
            # per PSUM tile so the PSUM->SBUF convert-copies are wide) ----
            def load_transposed(src_view, n_tiles, dst):
                for j0 in range(0, n_tiles, LB):
                    st = stage_pool.tile([128, LB, 128], F32, tag="stage")
                    nc.sync.dma_start(out=st[:], in_=src_view[:, j0 : j0 + LB, :])
                    for i0 in range(0, LB, 8):
                        nb = min(8, LB - i0)
                        pst = spsum.tile([128, 1024], F32, tag="s")
                        for i in range(nb):
                            nc.tensor.transpose(
                                pst[:, (i * 128) : (i + 1) * 128],
                                st[:, i0 + i, :],
                                identity[:],
                            )
                        nc.vector.tensor_copy(
                            dst[:, (j0 + i0) * 128 : (j0 + i0 + nb) * 128],
                            pst[:, : nb * 128],
                        )

            load_transposed(qv, NSC, qT)
            load_transposed(kv, NTT, kT)

            # ---- load v (convert f32 -> bf16 into the 129-strided layout) ----
            for j0 in range(0, NSC, LB):
                st = stage_pool.tile([128, LB, 128], F32, tag="stage")
                nc.sync.dma_start(out=st[:], in_=vw[:, j0 : j0 + LB, :])
                nc.vector.tensor_copy(vv3[:, j0 : j0 + LB, 0:C], st[:])

            # ---- main loop: QK^T -> exp -> PV per s-group ----
            for g in range(NG):
                pT = pT_pool.tile([128, GSC * TL], BF16, tag="pT")
                for lc in range(GSC):
                    sc = g * GSC + lc
                    lhs = qT[:, sc * 128 : (sc + 1) * 128]
                    for h in range(2):  # two [128, 1024] halves of t
                        ps = spsum.tile([128, 1024], F32, tag="s")
                        for n in range(2):
                            t_off = h * 1024 + n * 512
                            nc.tensor.matmul(
                                ps[:, n * 512 : (n + 1) * 512],
                                lhs,
                                kT[:, t_off : t_off + 512],
                                start=True,
                                stop=True,
                            )
                        nc.scalar.activation(
                            pT[:, lc * TL + h * 1024 : lc * TL + (h + 1) * 1024],
                            ps[:],
                            mybir.ActivationFunctionType.Exp,
                            scale=SCALE,
                        )
                for tt in range(NTT):
                    op = opsum.tile([128, C + 1], F32, tag="o")
                    for lc in range(GSC):
                        nc.tensor.matmul(
                            op[:],
                            pT[:, lc * TL + tt * 128 : lc * TL + (tt + 1) * 128],
                            vv3[:, g * GSC + lc, :],
                            start=(lc == 0),
                            stop=(lc == GSC - 1),
                        )
                    if g == 0:
                        nc.vector.tensor_copy(oacc3[:, tt, :], op[:])
                    else:
                        nc.vector.tensor_add(oacc3[:, tt, :], oacc3[:, tt, :], op[:])

            # ---- normalize + store ----
            for tt0 in range(0, NTT, 4):
                ost = ost_pool.tile([128, 4, 128], F32, tag="ost")
                for i in range(4):
                    tt = tt0 + i
                    nc.vector.reciprocal(
                        rtile[:, tt : tt + 1], oacc3[:, tt, C : C + 1]
                    )
                    nc.vector.tensor_scalar_mul(
                        ost[:, i, :], oacc3[:, tt, 0:C], rtile[:, tt : tt + 1]
                    )
                nc.sync.dma_start(out=ov[:, tt0 : tt0 + 4, :], in_=ost[:])

    nc.finalize()
    return nc


_NC_CACHE = None


def kernel(x: np.ndarray) -> np.ndarray:
    global _NC_CACHE
    x = np.asarray(x, dtype=np.float32)
    assert x.shape == (B, T, 3 * C), x.shape

    if _NC_CACHE is None:
        _NC_CACHE = build_nc()
    nc = _NC_CACHE

    in_maps = []
    for core in range(N_CORES):
        b, th = core // 2, core % 2
        in_maps.append(
            {
                "q": np.ascontiguousarray(x[b, :, 0:C]),
                "k": np.ascontiguousarray(x[b, th * TL : (th + 1) * TL, C : 2 * C]),
                "v": np.ascontiguousarray(x[b, :, 2 * C : 3 * C]),
            }
        )

    res = run_bass_kernel_spmd(nc, in_maps, core_ids=list(range(N_CORES)))

    out = np.empty((B, T, C), dtype=np.float32)
    for core in range(N_CORES):
        b, th = core // 2, core % 2
        out[b, th * TL : (th + 1) * TL] = res.results[core]["out"]
    return out


# revision 5
# speedup vs baseline: 1.1244x; 1.1244x over previous
"""Trainium2 Bass kernel for nn_AttLayer_9972914061697 (sparse_attention).

Reference computation (jax):
    q, k, v = split(x, 3, axis=-1)              # x: [B=4, T=4096, 3C=384]
    score   = einsum('btc,bsc->bts', k, q) / sqrt(C)
    out     = softmax(score, -1) @ v            # [B, T, C=128]

Sharding: 8 cores = 4 batches x 2 T-halves (data parallel, zero comm).
Each core holds the full q, v of its batch plus its 2048-row k chunk and
produces its 2048-row output chunk.

Per-core algorithm (all matmuls bf16, accumulation f32):
  - transpose q, k on-chip (PE transpose) into [C, T] layout
  - S_T[s, t] = sum_c q[s,c] k[t,c] computed as qT_chunk.T @ kT (PSUM f32)
  - P_T = exp(S_T / sqrt(C)) via ScalarE, written bf16 to SBUF
  - out[t, 0:128] & rowsum[t] in one PSUM accumulation: P_T_chunk.T @ [v | 1]
    (ones column appended to v makes the softmax denominator an extra column)
  - out = out * 1/rowsum (VectorE reciprocal + per-partition scalar mul)
"""

import numpy as np

import concourse.bass as bass
import concourse.tile as tile
from concourse import bacc, mybir
from concourse.bass_utils import run_bass_kernel_spmd
from concourse.masks import make_identity

F32 = mybir.dt.float32
BF16 = mybir.dt.bfloat16

B = 4
T = 4096
C = 128
N_CORES = 8
TL = T // 2          # 2048 t-rows per core
NSC = T // 128       # 32 s-chunks
NTT = TL // 128      # 16 t-tiles
GSC = 4              # s-chunks per group (PV flush granularity)
NG = NSC // GSC      # 8 groups
SCALE = 1.0 / float(np.sqrt(C))
LB = 4               # 128-row tiles per input DMA


def build_nc():
    nc = bacc.Bacc()
    q = nc.declare_dram_parameter("q", [T, C], F32, isOutput=False)
    k = nc.declare_dram_parameter("k", [TL, C], F32, isOutput=False)
    v = nc.declare_dram_parameter("v", [T, C], F32, isOutput=False)
    out = nc.declare_dram_parameter("out", [TL, C], F32, isOutput=True)

    qv = q[:].rearrange("(n p) c -> p n c", p=128)    # [128, 32, 128]
    kv = k[:].rearrange("(n p) c -> p n c", p=128)    # [128, 16, 128]
    vw = v[:].rearrange("(n p) c -> p n c", p=128)    # [128, 32, 128]
    ov = out[:].rearrange("(n p) c -> p n c", p=128)  # [128, 16, 128]

    with tile.TileContext(nc) as tc:
        with (
            tc.tile_pool(name="const", bufs=1) as const_pool,
            tc.tile_pool(name="qkt", bufs=1) as qkt_pool,
            tc.tile_pool(name="vbuf", bufs=1) as v_pool,
            tc.tile_pool(name="stage", bufs=4) as stage_pool,
            tc.tile_pool(name="pT", bufs=3) as pT_pool,
            tc.tile_pool(name="oacc", bufs=1) as oacc_pool,
            tc.tile_pool(name="ost", bufs=2) as ost_pool,
            tc.tile_pool(name="spsum", bufs=3, space="PSUM") as spsum,
            tc.tile_pool(name="opsum", bufs=2, space="PSUM") as opsum,
        ):
            identity = const_pool.tile([128, 128], F32)
            make_identity(nc, identity[:])

            qT = qkt_pool.tile([128, T], BF16, tag="qT")     # q transposed [c, s]
            kT = qkt_pool.tile([128, TL], BF16, tag="kT")    # k transposed [c, t]
            vv = v_pool.tile([128, NSC * (C + 1)], BF16)     # 32 x [128, 129] chunks
            vv3 = vv[:].rearrange("p (n c) -> p n c", c=C + 1)
            oacc = oacc_pool.tile([128, NTT * (C + 1)], F32)
            oacc3 = oacc[:].rearrange("p (n c) -> p n c", c=C + 1)
            rtile = const_pool.tile([128, NTT], F32, tag="recip")

            # ones column of every v chunk
            nc.vector.memset(vv3[:, :, C : C + 1], 1.0)

            # warm up the ACT exp table early so the ~2.7us table load
            # overlaps the prologue DMA instead of stalling the first score
            warm = const_pool.tile([128, 8], F32, tag="warm")
            nc.vector.memset(warm[:], 0.0)
            nc.scalar.activation(
                warm[:], warm[:], mybir.ActivationFunctionType.Exp, scale=1.0
            )

            # ---- load + transpose q into qT, k into kT (groups of 8 tiles
            # per PSUM tile so the PSUM->SBUF convert-copies are wide) ----
            def load_transposed(src_view, n_tiles, dst):
                for j0 in range(0, n_tiles, LB):
                    st = stage_pool.tile([128, LB, 128], F32, tag="stage")
                    nc.sync.dma_start(out=st[:], in_=src_view[:, j0 : j0 + LB, :])
                    for i0 in range(0, LB, 8):
                        nb = min(8, LB - i0)
                        pst = spsum.tile([128, 1024], F32, tag="s")
                        for i in range(nb):
                            nc.tensor.transpose(
                                pst[:, (i * 128) : (i + 1) * 128],
                                st[:, i0 + i, :],
                                identity[:],
                            )
                        nc.vector.tensor_copy(
                            dst[:, (j0 + i0) * 128 : (j0 + i0 + nb) * 128],
                            pst[:, : nb * 128],
                        )

            load_transposed(kv, NTT, kT)
            load_transposed(qv, NSC, qT)

            # ---- load v (convert f32 -> bf16 into the 129-strided layout) ----
            for j0 in range(0, NSC, LB):
                st = stage_pool.tile([128, LB, 128], F32, tag="stage")
                nc.sync.dma_start(out=st[:], in_=vw[:, j0 : j0 + LB, :])
                nc.vector.tensor_copy(vv3[:, j0 : j0 + LB, 0:C], st[:])

            # ---- main loop: QK^T -> exp -> PV per s-group ----
            for g in range(NG):
                pT = pT_pool.tile([128, GSC * TL], BF16, tag="pT")
                for lc in range(GSC):
                    sc = g * GSC + lc
                    lhs = qT[:, sc * 128 : (sc + 1) * 128]
                    for h in range(2):  # two [128, 1024] halves of t
                        ps = spsum.tile([128, 1024], F32, tag="s")
                        for n in range(2):
                            t_off = h * 1024 + n * 512
                            nc.tensor.matmul(
                                ps[:, n * 512 : (n + 1) * 512],
                                lhs,
                                kT[:, t_off : t_off + 512],
                                start=True,
                                stop=True,
                            )
                        nc.scalar.activation(
                            pT[:, lc * TL + h * 1024 : lc * TL + (h + 1) * 1024],
                            ps[:],
                            mybir.ActivationFunctionType.Exp,
                            scale=SCALE,
                        )
                for tt in range(NTT):
                    op = opsum.tile([128, C + 1], F32, tag="o")
                    for lc in range(GSC):
                        nc.tensor.matmul(
                            op[:],
                            pT[:, lc * TL + tt * 128 : lc * TL + (tt + 1) * 128],
                            vv3[:, g * GSC + lc, :],
                            start=(lc == 0),
                            stop=(lc == GSC - 1),
                        )
                    if g == 0:
                        nc.vector.tensor_copy(oacc3[:, tt, :], op[:])
                    else:
                        nc.vector.tensor_add(oacc3[:, tt, :], oacc3[:, tt, :], op[:])

            # ---- normalize + store ----
            for tt0 in range(0, NTT, 4):
                ost = ost_pool.tile([128, 4, 128], F32, tag="ost")
                for i in range(4):
                    tt = tt0 + i
                    nc.vector.reciprocal(
                        rtile[:, tt : tt + 1], oacc3[:, tt, C : C + 1]
                    )
                    nc.vector.tensor_scalar_mul(
                        ost[:, i, :], oacc3[:, tt, 0:C], rtile[:, tt : tt + 1]
                    )
                nc.sync.dma_start(out=ov[:, tt0 : tt0 + 4, :], in_=ost[:])

    nc.finalize()
    return nc


_NC_CACHE = None


def kernel(x: np.ndarray) -> np.ndarray:
    global _NC_CACHE
    x = np.asarray(x, dtype=np.float32)
    assert x.shape == (B, T, 3 * C), x.shape

    if _NC_CACHE is None:
        _NC_CACHE = build_nc()
    nc = _NC_CACHE

    in_maps = []
    for core in range(N_CORES):
        b, th = core // 2, core % 2
        in_maps.append(
            {
                "q": np.ascontiguousarray(x[b, :, 0:C]),
                "k": np.ascontiguousarray(x[b, th * TL : (th + 1) * TL, C : 2 * C]),
                "v": np.ascontiguousarray(x[b, :, 2 * C : 3 * C]),
            }
        )

    res = run_bass_kernel_spmd(nc, in_maps, core_ids=list(range(N_CORES)))

    out = np.empty((B, T, C), dtype=np.float32)
    for core in range(N_CORES):
        b, th = core // 2, core % 2
        out[b, th * TL : (th + 1) * TL] = res.results[core]["out"]
    return out


# revision 6
# speedup vs baseline: 1.1797x; 1.0492x over previous
"""Trainium2 Bass kernel for nn_AttLayer_9972914061697 (sparse_attention).

Reference computation (jax):
    q, k, v = split(x, 3, axis=-1)              # x: [B=4, T=4096, 3C=384]
    score   = einsum('btc,bsc->bts', k, q) / sqrt(C)
    out     = softmax(score, -1) @ v            # [B, T, C=128]

Sharding: 8 cores = 4 batches x 2 T-halves (data parallel, zero comm).
Each core holds the full q, v of its batch plus its 2048-row k chunk and
produces its 2048-row output chunk.

Per-core algorithm (matmuls bf16, accumulation f32):
  - transpose q, k on-chip (PE transpose) into [C, T] layout
  - S_T[s, t] = sum_c q[s,c] k[t,c] computed as qT_chunk.T @ kT (PSUM f32)
  - P_T = exp(S_T / sqrt(C)) via ScalarE, written bf16 to SBUF
  - out[t, 0:128] & rowsum[t] in one PSUM accumulation: P_T_chunk.T @ [v | 1]
    (ones column appended to v makes the softmax denominator an extra column)
  - out = out * 1/rowsum (VectorE reciprocal + per-partition scalar mul)

The s axis is processed in groups of GSC 128-row chunks; the q/v loads for
group g+2 are issued inside group g's body so DMA/transpose latency hides
behind compute, and each group's PV flush accumulates into SBUF via VectorE.
"""

import numpy as np

import concourse.bass as bass
import concourse.tile as tile
from concourse import bacc, mybir
from concourse.bass_utils import run_bass_kernel_spmd
from concourse.masks import make_identity

F32 = mybir.dt.float32
BF16 = mybir.dt.bfloat16

B = 4
T = 4096
C = 128
N_CORES = 8
TL = T // 2          # 2048 t-rows per core
NSC = T // 128       # 32 s-chunks
NTT = TL // 128      # 16 t-tiles
GSC = 4              # s-chunks per group (PV flush granularity)
NG = NSC // GSC      # 8 groups
SCALE = 1.0 / float(np.sqrt(C))
LB = 4               # 128-row tiles per input DMA batch


def build_nc():
    nc = bacc.Bacc()
    q = nc.declare_dram_parameter("q", [T, C], F32, isOutput=False)
    k = nc.declare_dram_parameter("k", [TL, C], F32, isOutput=False)
    v = nc.declare_dram_parameter("v", [T, C], F32, isOutput=False)
    out = nc.declare_dram_parameter("out", [TL, C], F32, isOutput=True)

    qv = q[:].rearrange("(n p) c -> p n c", p=128)    # [128, 32, 128]
    kv = k[:].rearrange("(n p) c -> p n c", p=128)    # [128, 16, 128]
    vw = v[:].rearrange("(n p) c -> p n c", p=128)    # [128, 32, 128]
    ov = out[:].rearrange("(n p) c -> p n c", p=128)  # [128, 16, 128]

    with tile.TileContext(nc) as tc:
        with (
            tc.tile_pool(name="const", bufs=1) as const_pool,
            tc.tile_pool(name="qkt", bufs=1) as qkt_pool,
            tc.tile_pool(name="vbuf", bufs=1) as v_pool,
            tc.tile_pool(name="stage", bufs=6) as stage_pool,
            tc.tile_pool(name="pT", bufs=3) as pT_pool,
            tc.tile_pool(name="oacc", bufs=1) as oacc_pool,
            tc.tile_pool(name="ost", bufs=2) as ost_pool,
            tc.tile_pool(name="spsum", bufs=3, space="PSUM") as spsum,
            tc.tile_pool(name="opsum", bufs=2, space="PSUM") as opsum,
        ):
            identity = const_pool.tile([128, 128], F32)
            make_identity(nc, identity[:])

            qT = qkt_pool.tile([128, T], BF16, tag="qT")     # q transposed [c, s]
            kT = qkt_pool.tile([128, TL], BF16, tag="kT")    # k transposed [c, t]
            vv = v_pool.tile([128, NSC * (C + 1)], BF16)     # 32 x [128, 129] chunks
            vv3 = vv[:].rearrange("p (n c) -> p n c", c=C + 1)
            oacc = oacc_pool.tile([128, NTT * (C + 1)], F32)
            oacc3 = oacc[:].rearrange("p (n c) -> p n c", c=C + 1)
            rtile = const_pool.tile([128, NTT], F32, tag="recip")

            # ones column of every v chunk
            nc.vector.memset(vv3[:, :, C : C + 1], 1.0)

            # warm up the ACT exp table early so the ~2.7us table load
            # overlaps the prologue DMA instead of stalling the first score
            warm = const_pool.tile([128, 8], F32, tag="warm")
            nc.vector.memset(warm[:], 0.0)
            nc.scalar.activation(
                warm[:], warm[:], mybir.ActivationFunctionType.Exp, scale=1.0
            )

            def load_transposed(src_view, j0, n_tiles, dst):
                """DMA n_tiles 128-row tiles starting at tile j0 and write
                their transpose into dst[:, j0*128 : (j0+n_tiles)*128]."""
                st = stage_pool.tile([128, n_tiles, 128], F32, tag="stage")
                nc.sync.dma_start(out=st[:], in_=src_view[:, j0 : j0 + n_tiles, :])
                for i0 in range(0, n_tiles, 8):
                    nb = min(8, n_tiles - i0)
                    pst = spsum.tile([128, 1024], F32, tag="s")
                    for i in range(nb):
                        nc.tensor.transpose(
                            pst[:, (i * 128) : (i + 1) * 128],
                            st[:, i0 + i, :],
                            identity[:],
                        )
                    nc.vector.tensor_copy(
                        dst[:, (j0 + i0) * 128 : (j0 + i0 + nb) * 128],
                        pst[:, : nb * 128],
                    )

            def load_v(j0, n_tiles):
                st = stage_pool.tile([128, n_tiles, 128], F32, tag="stage")
                nc.sync.dma_start(out=st[:], in_=vw[:, j0 : j0 + n_tiles, :])
                nc.gpsimd.tensor_copy(vv3[:, j0 : j0 + n_tiles, 0:C], st[:])

            # k fully up front (QK needs all of kT), then the first two
            # groups' worth of q and v; the rest stream inside the main loop.
            for j0 in range(0, NTT, LB):
                load_transposed(kv, j0, LB, kT)
            load_transposed(qv, 0, LB, qT)
            load_transposed(qv, LB, LB, qT)
            load_v(0, LB)
            load_v(LB, LB)

            # ---- main loop over s-groups: QK^T -> exp -> PV ----
            for g in range(NG):
                if g + 2 < NG:
                    load_transposed(qv, (g + 2) * GSC, GSC, qT)
                    load_v((g + 2) * GSC, GSC)
                pT = pT_pool.tile([128, GSC * TL], BF16, tag="pT")
                for lc in range(GSC):
                    sc = g * GSC + lc
                    lhs = qT[:, sc * 128 : (sc + 1) * 128]
                    for h in range(2):  # two [128, 1024] halves of t
                        ps = spsum.tile([128, 1024], F32, tag="s")
                        for n in range(2):
                            t_off = h * 1024 + n * 512
                            nc.tensor.matmul(
                                ps[:, n * 512 : (n + 1) * 512],
                                lhs,
                                kT[:, t_off : t_off + 512],
                                start=True,
                                stop=True,
                            )
                        nc.scalar.activation(
                            pT[:, lc * TL + h * 1024 : lc * TL + (h + 1) * 1024],
                            ps[:],
                            mybir.ActivationFunctionType.Exp,
                            scale=SCALE,
                        )
                for tt2 in range(NTT // 2):  # pairs of t-tiles per PSUM bank
                    op = opsum.tile([128, 2 * (C + 1)], F32, tag="o")
                    for half in range(2):
                        tt = tt2 * 2 + half
                        for lc in range(GSC):
                            nc.tensor.matmul(
                                op[:, half * (C + 1) : (half + 1) * (C + 1)],
                                pT[:, lc * TL + tt * 128 : lc * TL + (tt + 1) * 128],
                                vv3[:, g * GSC + lc, :],
                                start=(lc == 0),
                                stop=(lc == GSC - 1),
                            )
                    dst = oacc[:, tt2 * 2 * (C + 1) : (tt2 + 1) * 2 * (C + 1)]
                    if g == 0:
                        nc.vector.tensor_copy(dst, op[:])
                    else:
                        nc.vector.tensor_add(dst, dst, op[:])

            # ---- normalize + store ----
            for tt0 in range(0, NTT, 4):
                ost = ost_pool.tile([128, 4, 128], F32, tag="ost")
                for i in range(4):
                    tt = tt0 + i
                    nc.vector.reciprocal(
                        rtile[:, tt : tt + 1], oacc3[:, tt, C : C + 1]
                    )
                    nc.vector.tensor_scalar_mul(
                        ost[:, i, :], oacc3[:, tt, 0:C], rtile[:, tt : tt + 1]
                    )
                nc.sync.dma_start(out=ov[:, tt0 : tt0 + 4, :], in_=ost[:])

    nc.finalize()
    return nc


_NC_CACHE = None


def kernel(x: np.ndarray) -> np.ndarray:
    global _NC_CACHE
    x = np.asarray(x, dtype=np.float32)
    assert x.shape == (B, T, 3 * C), x.shape

    if _NC_CACHE is None:
        _NC_CACHE = build_nc()
    nc = _NC_CACHE

    in_maps = []
    for core in range(N_CORES):
        b, th = core // 2, core % 2
        in_maps.append(
            {
                "q": np.ascontiguousarray(x[b, :, 0:C]),
                "k": np.ascontiguousarray(x[b, th * TL : (th + 1) * TL, C : 2 * C]),
                "v": np.ascontiguousarray(x[b, :, 2 * C : 3 * C]),
            }
        )

    res = run_bass_kernel_spmd(nc, in_maps, core_ids=list(range(N_CORES)))

    out = np.empty((B, T, C), dtype=np.float32)
    for core in range(N_CORES):
        b, th = core // 2, core % 2
        out[b, th * TL : (th + 1) * TL] = res.results[core]["out"]
    return out


# revision 9
# speedup vs baseline: 1.3907x; 1.1789x over previous
"""Trainium2 Bass kernel for nn_AttLayer_9972914061697 (sparse_attention).

Reference computation (jax):
    q, k, v = split(x, 3, axis=-1)              # x: [B=4, T=4096, 3C=384]
    score   = einsum('btc,bsc->bts', k, q) / sqrt(C)
    out     = softmax(score, -1) @ v            # [B, T, C=128]

Sharding: 8 cores = 4 batches x 2 T-halves (data parallel, zero comm).
Each core holds the full q, v of its batch plus its 2048-row k chunk and
produces its 2048-row output chunk. q/k/v are shipped as bf16 (the matmul
compute dtype; identical numerics to an on-device cast), output is f32.

Per-core algorithm (matmuls bf16, accumulation f32):
  - q, k land transposed in SBUF via XBAR DMA-transpose ([C, T] layout)
  - S_T[s, t] = sum_c q[s,c] k[t,c] computed as qT_chunk.T @ kT (PSUM f32)
  - P_T = exp(S_T / sqrt(C)) via ScalarE, written bf16 to SBUF
  - out[t, 0:128] & rowsum[t] in one PSUM accumulation: P_T_chunk.T @ [v | 1]
    (ones column appended to v makes the softmax denominator an extra column)
  - out = out * 1/rowsum (VectorE reciprocal + per-partition scalar mul)

The s axis runs in groups of GSC 128-row chunks, software-pipelined:
group g's QK+exp is emitted before group g-1's PV so the ScalarE exp stream
never starves, and loads for later groups are issued alongside.
"""

import numpy as np
import ml_dtypes

import concourse.bass as bass
import concourse.tile as tile
from concourse import bacc, mybir
from concourse.bass_utils import run_bass_kernel_spmd

F32 = mybir.dt.float32
BF16 = mybir.dt.bfloat16

B = 4
T = 4096
C = 128
N_CORES = 8
TL = T // 2          # 2048 t-rows per core
NSC = T // 128       # 32 s-chunks
NTT = TL // 128      # 16 t-tiles
GSC = 4              # s-chunks per group (PV flush granularity)
NG = NSC // GSC      # 8 groups
GW = GSC * 128       # s-rows per group (512)
SCALE = 1.0 / float(np.sqrt(C))


def build_nc():
    nc = bacc.Bacc()
    q = nc.declare_dram_parameter("q", [T, C], BF16, isOutput=False)
    k = nc.declare_dram_parameter("k", [TL, C], BF16, isOutput=False)
    v = nc.declare_dram_parameter("v", [T, C], BF16, isOutput=False)
    out = nc.declare_dram_parameter("out", [TL, C], F32, isOutput=True)

    vw = v[:].rearrange("(n p) c -> p n c", p=128)    # [128, 32, 128]
    ov = out[:].rearrange("(n p) c -> p n c", p=128)  # [128, 16, 128]

    with tile.TileContext(nc) as tc:
        with (
            tc.tile_pool(name="const", bufs=1) as const_pool,
            tc.tile_pool(name="qkt", bufs=1) as qkt_pool,
            tc.tile_pool(name="vbuf", bufs=1) as v_pool,
            tc.tile_pool(name="pT", bufs=3) as pT_pool,
            tc.tile_pool(name="oacc", bufs=1) as oacc_pool,
            tc.tile_pool(name="ost", bufs=2) as ost_pool,
            tc.tile_pool(name="spsum", bufs=3, space="PSUM") as spsum,
            tc.tile_pool(name="opsum", bufs=2, space="PSUM") as opsum,
        ):
            qT = qkt_pool.tile([128, T], BF16, tag="qT")     # q transposed [c, s]
            kT = qkt_pool.tile([128, TL], BF16, tag="kT")    # k transposed [c, t]
            vv = v_pool.tile([128, NSC * (C + 1)], BF16)     # 32 x [128, 129] chunks
            vv3 = vv[:].rearrange("p (n c) -> p n c", c=C + 1)
            oacc = oacc_pool.tile([128, NTT * (C + 1)], F32)
            oacc3 = oacc[:].rearrange("p (n c) -> p n c", c=C + 1)
            rtile = const_pool.tile([128, NTT], F32, tag="recip")

            # ones column of every v chunk
            nc.vector.memset(vv3[:, :, C : C + 1], 1.0)

            # warm up the ACT exp table early so the ~2.7us table load
            # overlaps the prologue DMA instead of stalling the first score
            warm = const_pool.tile([128, 8], F32, tag="warm")
            nc.vector.memset(warm[:], 0.0)
            nc.scalar.activation(
                warm[:], warm[:], mybir.ActivationFunctionType.Exp, scale=1.0
            )

            def load_qT(g):
                nc.sync.dma_start(
                    out=qT[:, g * GW : (g + 1) * GW],
                    in_=q[g * GW : (g + 1) * GW, :],
                    transpose=True,
                )

            def load_v(g):
                nc.sync.dma_start(
                    out=vv3[:, g * GSC : (g + 1) * GSC, 0:C],
                    in_=vw[:, g * GSC : (g + 1) * GSC, :],
                )

            for j in range(4):
                nc.sync.dma_start(
                    out=kT[:, j * 512 : (j + 1) * 512],
                    in_=k[j * 512 : (j + 1) * 512, :],
                    transpose=True,
                )
            load_qT(0)
            load_qT(1)
            load_v(0)

            def qk_exp_group(g, pT):
                for lc in range(GSC):
                    sc = g * GSC + lc
                    lhs = qT[:, sc * 128 : (sc + 1) * 128]
                    for h in range(2):  # two [128, 1024] halves of t
                        ps = spsum.tile([128, 1024], F32, tag="s")
                        for n in range(2):
                            t_off = h * 1024 + n * 512
                            nc.tensor.matmul(
                                ps[:, n * 512 : (n + 1) * 512],
                                lhs,
                                kT[:, t_off : t_off + 512],
                                start=True,
                                stop=True,
                            )
                        nc.scalar.activation(
                            pT[:, lc * TL + h * 1024 : lc * TL + (h + 1) * 1024],
                            ps[:],
                            mybir.ActivationFunctionType.Exp,
                            scale=SCALE,
                        )

            def pv_group(g, pT, final):
                ost = None
                for tt2 in range(NTT // 2):  # pairs of t-tiles per PSUM bank
                    op = opsum.tile([128, 2 * (C + 1)], F32, tag="o")
                    for half in range(2):
                        tt = tt2 * 2 + half
                        for lc in range(GSC):
                            nc.tensor.matmul(
                                op[:, half * (C + 1) : (half + 1) * (C + 1)],
                                pT[:, lc * TL + tt * 128 : lc * TL + (tt + 1) * 128],
                                vv3[:, g * GSC + lc, :],
                                start=(lc == 0),
                                stop=(lc == GSC - 1),
                            )
                    dst = oacc[:, tt2 * 2 * (C + 1) : (tt2 + 1) * 2 * (C + 1)]
                    if g == 0:
                        nc.vector.tensor_copy(dst, op[:])
                    else:
                        nc.vector.tensor_add(dst, dst, op[:])
                    if final:
                        # normalize + store as soon as each t-tile pair is done
                        if tt2 % 2 == 0:
                            ost = ost_pool.tile([128, 4, 128], F32, tag="ost")
                        for half in range(2):
                            tt = tt2 * 2 + half
                            nc.vector.reciprocal(
                                rtile[:, tt : tt + 1], oacc3[:, tt, C : C + 1]
                            )
                            nc.vector.tensor_scalar_mul(
                                ost[:, (tt2 % 2) * 2 + half, :],
                                oacc3[:, tt, 0:C],
                                rtile[:, tt : tt + 1],
                            )
                        if tt2 % 2 == 1:
                            tt0 = (tt2 - 1) * 2
                            nc.sync.dma_start(
                                out=ov[:, tt0 : tt0 + 4, :], in_=ost[:]
                            )

            # ---- software-pipelined main loop ----
            pT_tiles = {}
            for g in range(NG):
                if g + 2 < NG:
                    load_qT(g + 2)
                if g + 1 < NG:
                    load_v(g + 1)
                pT_g = pT_pool.tile([128, GSC * TL], BF16, tag="pT")
                pT_tiles[g] = pT_g
                qk_exp_group(g, pT_tiles[g])
                if g >= 1:
                    pv_group(g - 1, pT_tiles[g - 1], final=False)
                    del pT_tiles[g - 1]
            pv_group(NG - 1, pT_tiles[NG - 1], final=True)

    nc.finalize()
    return nc


_NC_CACHE = None


def make_in_maps(x: np.ndarray):
    xb = np.asarray(x, dtype=np.float32).astype(ml_dtypes.bfloat16)
    in_maps = []
    for core in range(N_CORES):
        b, th = core // 2, core % 2
        in_maps.append(
            {
                "q": np.ascontiguousarray(xb[b, :, 0:C]),
                "k": np.ascontiguousarray(xb[b, th * TL : (th + 1) * TL, C : 2 * C]),
                "v": np.ascontiguousarray(xb[b, :, 2 * C : 3 * C]),
            }
        )
    return in_maps


def kernel(x: np.ndarray) -> np.ndarray:
    global _NC_CACHE
    x = np.asarray(x, dtype=np.float32)
    assert x.shape == (B, T, 3 * C), x.shape

    if _NC_CACHE is None:
        _NC_CACHE = build_nc()
    nc = _NC_CACHE

    res = run_bass_kernel_spmd(nc, make_in_maps(x), core_ids=list(range(N_CORES)))

    out = np.empty((B, T, C), dtype=np.float32)
    for core in range(N_CORES):
        b, th = core // 2, core % 2
        out[b, th * TL : (th + 1) * TL] = res.results[core]["out"]
    return out
